# revision 25
# baseline (speedup 1.0000x reference)
"""Distributed GQA attention block (dense_transformer) for 8 TRN2 NeuronCores.

Sharding: Megatron-style head sharding for QKV+attention (each core owns 4 Q
heads / 1 KV head), Ulysses-style AllToAll to switch to sequence sharding for
the output projection (each core owns 256 rows per batch).

v2 scheduling rewrite vs baseline:
  - Pinned ACT table (natural_log_exp_and_others) -> zero table reloads;
    rstd = exp(-0.5 ln(m)), softmax exp, all one table.
  - Softmax denominators gathered partition-major [16,256] and inverted with
    reciprocal_approx_fast (f32) instead of a 13us DVE InstReciprocal.
  - Projections software-pipelined: 16-matmul bursts never wait on the
    norm/rope chain; the small norm matmuls (ssp/bcs/swps) of chunk r are
    emitted between the bursts of chunk r+1.
  - PSUM evacuated immediately (qraw copies) so the 2-slot ring never stalls.
  - x laid out chunk-contiguous on host -> one big DMA per chunk.
  - wo resident in SBUF (loaded once), q/k scale folded into the broadcast
    matmul weights.
  - b1 output projection split into even-k pass (overlapped with b1/hp1
    attention, after a2a(1,0)) and odd-k tail pass (after a2a(1,1)).
Compute in bf16 on the TensorEngine (f32 accumulation), f32 softmax.
"""
import sys

if '/opt/trn_rl_repo' not in sys.path:
    sys.path.insert(0, '/opt/trn_rl_repo')

import numpy as np
import ml_dtypes

N_CORES = 8
B, S, D = 2, 2048, 2048
DH = 64
ROWS = B * S        # 4096
NKT = D // 128      # 16 contraction tiles
RC = 512            # row chunk
NCHUNK = ROWS // RC # 8
EPS = 1e-6
ACT_TABLE_LN_EXP = 6  # natural_log_exp_and_others in act_info.json

_cache = {}


def _build(_DEBUG=False):
    import concourse.mybir as mybir
    import concourse.tile as tile
    from concourse import bacc
    from concourse.bass import ts, ds

    f32 = mybir.dt.float32
    bf = mybir.dt.bfloat16
    AF = mybir.ActivationFunctionType

    nc = bacc.Bacc()
    x_sb = nc.declare_dram_parameter("x_sb", [128, NCHUNK * NKT * RC], bf, isOutput=False)
    wq_sb = nc.declare_dram_parameter("wq_sb", [128, NKT * 2 * 128], bf, isOutput=False)
    wkv_sb = nc.declare_dram_parameter("wkv_sb", [128, NKT * 128], bf, isOutput=False)
    wo_sb = nc.declare_dram_parameter("wo_sb", [128, 16 * 16 * 128], bf, isOutput=False)
    cos_sb = nc.declare_dram_parameter("cos_sb", [128, ROWS], bf, isOutput=False)
    sin_sb = nc.declare_dram_parameter("sin_sb", [128, ROWS], bf, isOutput=False)
    tri_sb = nc.declare_dram_parameter("tri_sb", [128, 128], bf, isOutput=False)
    ind_sb = nc.declare_dram_parameter("ind_sb", [128, 2], bf, isOutput=False)
    indtq_sb = nc.declare_dram_parameter("indtq_sb", [2, 128], bf, isOutput=False)
    indtk_sb = nc.declare_dram_parameter("indtk_sb", [1, 64], bf, isOutput=False)
    selj_sb = nc.declare_dram_parameter("selj_sb", [16, 8 * 128], bf, isOutput=False)
    idn_sb = nc.declare_dram_parameter("idn_sb", [64, 64], bf, isOutput=False)
    psw_sb = nc.declare_dram_parameter("psw_sb", [128, 128], bf, isOutput=False)
    out_ext = nc.declare_dram_parameter("out", [D, 512], f32, isOutput=True)
    dbg_ext = nc.declare_dram_parameter("dbg", [128, 16 * 65 + 4 * 256 + 16 * 256 + ROWS], bf, isOutput=True) if _DEBUG else None
    dbg2_ext = nc.declare_dram_parameter("dbg2", [128, 2 * 2080 + 512], bf, isOutput=True) if _DEBUG else None

    with tile.TileContext(nc) as tc:
        with (
            tc.tile_pool(name="cp", bufs=1) as cp,
            tc.tile_pool(name="xp", bufs=2) as xp,
            tc.tile_pool(name="sp", bufs=2) as sp,
            tc.tile_pool(name="ep", bufs=2) as ep,
            tc.tile_pool(name="dram", bufs=1, space="DRAM") as dram,
            tc.tile_pool(name="mm", bufs=2, space="PSUM") as mmp,
            tc.tile_pool(name="sm", bufs=2, space="PSUM") as smp,
            tc.tile_pool(name="po", bufs=2, space="PSUM") as po,
        ):
            # ---- pin the single ACT table (ln+exp+copy) ----
            ld = mybir.InstLoadActFuncSet(
                name=nc.get_next_instruction_name(), ins=[], outs=[],
                act_func_set_id=ACT_TABLE_LN_EXP)
            nc.scalar.add_instruction(ld)

            # ---- persistent constants (DMAs emitted in the schedule) ----
            wq = cp.tile([128, NKT * 2 * 128], bf)
            wkvt = cp.tile([128, NKT * 128], bf)
            tri = cp.tile([128, 128], bf)
            ind = cp.tile([128, 2], bf)
            indtq = cp.tile([2, 128], bf)
            indtk = cp.tile([1, 64], bf)
            selj = cp.tile([16, 8 * 128], bf)
            idn = cp.tile([64, 64], bf)
            pswap = cp.tile([128, 128], bf)
            epsc = cp.tile([128, 1], f32)
            nc.gpsimd.memset(epsc[:], EPS)

            wo = cp.tile([128, 16 * 16 * 128], bf)  # resident, loaded in 4 parts

            QTn = [cp.tile([128, ROWS], bf, name=f"qtn{i}") for i in range(2)]
            KTd = [cp.tile([128, S], bf, name=f"ktd{b}") for b in range(B)]
            Vb1 = [cp.tile([128, 16 * 65], bf, name=f"vb{b}") for b in range(B)]
            attb = [cp.tile([128, 16 * 256], bf, name=f"attb{b}") for b in range(B)]
            OPART = cp.tile([128, 8 * 512], bf)  # b1 even-k oproj partials

            # ones columns of Vb1 (once)
            for b in range(B):
                nc.vector.memset(
                    Vb1[b][:].rearrange("p (j e) -> p j e", j=16)[:, :, 64:65], 1.0)

            a2a_in = [[dram.tile([1040, 256], bf, name=f"a2ain{b}{h}") for h in range(2)]
                      for b in range(B)]
            a2a_out = [[dram.tile([1040, 256], bf, name=f"a2aout{b}{h}") for h in range(2)]
                       for b in range(B)]

            # per-chunk sbuf state carried between pipeline stages
            st = [dict() for _ in range(NCHUNK)]

            x4 = x_sb[:].rearrange("p (r k q) -> p r k q", r=NCHUNK, k=NKT)

            def S0(r):  # prefetch chunk r inputs
                if r >= NCHUNK or "xt" in st[r]:
                    return
                xt = xp.tile([128, NKT, RC], bf, tag="xt")
                nc.sync.dma_start(xt[:], x4[:, r])
                cosc = sp.tile([128, RC], bf, tag="cos")
                nc.sync.dma_start(cosc[:], cos_sb[:, ds(r * RC, RC)])
                sinc = sp.tile([128, RC], bf, tag="sin")
                nc.sync.dma_start(sinc[:], sin_sb[:, ds(r * RC, RC)])
                st[r].update(xt=xt, cos=cosc, sin=sinc)

            def S1(r):  # Q head-pair 0 projection burst
                S0(r + 1)
                c = st[r]
                psq = mmp.tile([128, RC], f32, tag="mm", padded_shape=[128, 1024],
                               name=f"psq0_{r}")
                for k in range(NKT):
                    nc.tensor.matmul(psq[:], wq[:, ds(k * 128, 128)],
                                     c["xt"][:, k, :], start=(k == 0), stop=(k == NKT - 1))
                qraw = sp.tile([128, RC], bf, tag="qraw0")
                nc.scalar.activation(qraw[:], psq[:], AF.Copy)
                sq = sp.tile([128, RC], bf, tag="sq0")
                nc.vector.tensor_mul(sq[:], qraw[:], qraw[:])
                c.update(qraw0=qraw, sq0=sq)

            def S2(r):  # Q head-pair 1 projection burst
                c = st[r]
                psq = mmp.tile([128, RC], f32, tag="mm", padded_shape=[128, 1024],
                               name=f"psq1_{r}")
                for k in range(NKT):
                    nc.tensor.matmul(psq[:], wq[:, ds(2048 + k * 128, 128)],
                                     c["xt"][:, k, :], start=(k == 0), stop=(k == NKT - 1))
                qraw = sp.tile([128, RC], bf, tag="qraw1")
                nc.scalar.activation(qraw[:], psq[:], AF.Copy)
                sq = sp.tile([128, RC], bf, tag="sq1")
                nc.vector.tensor_mul(sq[:], qraw[:], qraw[:])
                c.update(qraw1=qraw, sq1=sq)

            def S3(r):  # KV projection burst
                c = st[r]
                pskv = mmp.tile([128, RC], f32, tag="mm", padded_shape=[128, 1024],
                                name=f"pskv_{r}")
                for k in range(NKT):
                    nc.tensor.matmul(pskv[:], wkvt[:, ts(k, 128)], c["xt"][:, k, :],
                                     start=(k == 0), stop=(k == NKT - 1))
                kraw = sp.tile([64, RC], bf, tag="kraw")
                nc.scalar.activation(kraw[:], pskv[0:64, :], AF.Copy)
                vtmp = sp.tile([64, RC], bf, tag="vtmp")
                nc.scalar.activation(vtmp[:], pskv[64:128, :], AF.Copy)
                sqk = sp.tile([64, RC], bf, tag="sqk")
                nc.vector.tensor_mul(sqk[:], kraw[:], kraw[:])
                c.update(kraw=kraw, vtmp=vtmp, sqk=sqk)

            def PS(r):  # sum-of-squares matmuls + rstd on ACT
                c = st[r]
                rstds = []
                for nm, sqt, indap, np_ in (("q0", c["sq0"], ind[:], 2),
                                            ("q1", c["sq1"], ind[:], 2),
                                            ("k", c["sqk"], ind[0:64, 0:1], 1)):
                    ssp = smp.tile([np_, RC], f32, tag="small", name=f"ssp{nm}_{r}")
                    nc.tensor.matmul(ssp[:], indap, sqt[:], start=True, stop=True)
                    lg = smp.tile([np_, RC], f32, tag="small", name=f"lg{nm}_{r}")
                    nc.scalar.activation(lg[:], ssp[:], AF.Ln, scale=1.0 / 64,
                                         bias=epsc[0:np_, :])
                    rstd = sp.tile([np_, RC], bf, tag=f"rstd{nm}", bufs=1)
                    nc.scalar.activation(rstd[:], lg[:], AF.Exp, scale=-0.5)
                    rstds.append(rstd)
                c.update(rstd0=rstds[0], rstd1=rstds[1], rstdk=rstds[2])

            def PB(r):  # broadcast rstd (scale-folded) + qn muls
                c = st[r]
                bcsq = mmp.tile([128, 2 * RC], f32, tag="mm", name=f"bcsq_{r}")
                nc.tensor.matmul(bcsq[:, 0:RC], indtq[:], c["rstd0"][:],
                                 start=True, stop=True)
                nc.tensor.matmul(bcsq[:, RC:2 * RC], indtq[:], c["rstd1"][:],
                                 start=True, stop=True)
                bcsk = smp.tile([64, RC], f32, tag="small", name=f"bcsk_{r}")
                nc.tensor.matmul(bcsk[:], indtk[:], c["rstdk"][:],
                                 start=True, stop=True)
                qn0 = sp.tile([128, RC], bf, tag="qn0", bufs=1)
                nc.vector.tensor_mul(qn0[:], c["qraw0"][:], bcsq[:, 0:RC])
                qn1 = sp.tile([128, RC], bf, tag="qn1", bufs=1)
                nc.vector.tensor_mul(qn1[:], c["qraw1"][:], bcsq[:, RC:2 * RC])
                kn = sp.tile([64, RC], bf, tag="kn", bufs=1)
                nc.vector.tensor_mul(kn[:], c["kraw"][:], bcsk[:])
                c.update(qn0=qn0, qn1=qn1, kn=kn)

            def PW(r):  # rope swap matmuls + V transposes + final rope writes
                c = st[r]
                b, sl = r // 4, r % 4
                swpq = mmp.tile([128, 2 * RC], f32, tag="mm", name=f"swpq_{r}")
                nc.tensor.matmul(swpq[:, 0:RC], pswap[:], c["qn0"][:],
                                 start=True, stop=True)
                nc.tensor.matmul(swpq[:, RC:2 * RC], pswap[:], c["qn1"][:],
                                 start=True, stop=True)
                swpk = smp.tile([64, RC], f32, tag="small", name=f"swpk_{r}")
                nc.tensor.matmul(swpk[:], pswap[0:64, 0:64], c["kn"][:],
                                 start=True, stop=True)
                tp = smp.tile([128, 4 * 64], bf, tag="small", name=f"tp_{r}")
                for t4 in range(4):
                    nc.tensor.transpose(tp[:, ts(t4, 64)],
                                        c["vtmp"][:, ts(t4, 128)], idn[:])
                # rope combine on DVE
                for hp in range(2):
                    qn = c["qn0"] if hp == 0 else c["qn1"]
                    sw = sp.tile([128, RC], bf, tag=f"sw{hp}", bufs=1)
                    nc.vector.tensor_mul(sw[:], swpq[:, ds(hp * RC, RC)], c["sin"][:])
                    qc = sp.tile([128, RC], bf, tag=f"qc{hp}", bufs=1)
                    nc.vector.tensor_mul(qc[:], qn[:], c["cos"][:])
                    nc.vector.tensor_add(QTn[hp][:, ds(r * RC, RC)], qc[:], sw[:])
                swk = sp.tile([64, RC], bf, tag="swk", bufs=1)
                nc.vector.tensor_mul(swk[:], swpk[:], c["sin"][0:64, :])
                kc = sp.tile([64, RC], bf, tag="kc", bufs=1)
                nc.vector.tensor_mul(kc[:], c["kn"][:], c["cos"][0:64, :])
                nc.vector.tensor_add(KTd[b][0:64, ds(sl * RC, RC)], kc[:], swk[:])
                nc.vector.tensor_add(KTd[b][64:128, ds(sl * RC, RC)], kc[:], swk[:])
                # V copy (4 transposed tiles -> 65-strided Vb1 columns)
                vdst = Vb1[b][:].rearrange("p (j e) -> p j e", j=16)[:, ds(sl * 4, 4), 0:64]
                nc.vector.tensor_copy(vdst, tp[:].rearrange("p (j e) -> p j e", j=4))
                st[r] = {}

            # ---- filler queue ----
            fill_q = []

            def fill(n=1):
                for _ in range(n):
                    if fill_q:
                        fill_q.pop(0)()

            # ---- attention block ----
            def attn_block(b, hp, qs):
                psO = [po.tile([65, RC], f32, tag="o", name=f"psO{b}{hp}{qs}{t}")
                       for t in range(2)]
                jmax = qs * 4 + 3

                def scores(j):
                    dj = j - qs * 4
                    p = dj * 128 if dj >= 0 else 0
                    N = RC - p
                    qb = b * S + qs * RC + p
                    psS = mmp.tile([128, 2 * RC], f32, tag="mm", name="psS")
                    nc.tensor.matmul(psS[:, 0:N], KTd[b][0:64, ts(j, 128)],
                                     QTn[hp][0:64, ds(qb, N)], start=True, stop=True,
                                     tile_position=(0, 0))
                    nc.tensor.matmul(psS[:, RC:RC + N], KTd[b][64:128, ts(j, 128)],
                                     QTn[hp][64:128, ds(qb, N)], start=True, stop=True,
                                     tile_position=(64, 0))
                    E = ep.tile([128, 2 * RC], bf, tag="E")
                    e3 = E[:].rearrange("p (t q) -> p t q", t=2)
                    s3 = psS[:].rearrange("p (t q) -> p t q", t=2)
                    nc.scalar.activation(e3[:, :, p:RC], s3[:, :, 0:N], AF.Exp,
                                         scale=0.125)
                    if dj >= 0:
                        nc.vector.tensor_mul(
                            e3[:, :, ds(p, 128)], e3[:, :, ds(p, 128)],
                            tri[:].unsqueeze(1).broadcast_to([128, 2, 128]))
                    return p, E

                def av(j, pE):
                    p, E = pE
                    for t in range(2):
                        nc.tensor.matmul(psO[t][:, p:RC], Vb1[b][:, ds(j * 65, 65)],
                                         E[:, ds(t * RC + p, RC - p)],
                                         start=(j == 0), stop=(j == jmax))

                pend = {0: scores(0)}
                if jmax >= 1:
                    pend[1] = scores(1)
                for j in range(2, jmax + 1):
                    av(j - 2, pend.pop(j - 2))
                    pend[j] = scores(j)
                    if j % 3 == 2:
                        fill(1)
                for j in sorted(pend):
                    av(j, pend[j])
                for t in range(2):
                    on65 = sp.tile([65, RC], bf, tag="on")
                    if qs == 3:
                        nc.scalar.activation(on65[:], psO[t][:], AF.Copy)
                    else:
                        nc.vector.tensor_copy(on65[:], psO[t][:])
                    nc.sync.dma_start(
                        a2a_in[b][hp][ds(130 * (2 * qs) + 65 * t, 65), :], on65[:, 0:256])
                    nc.sync.dma_start(
                        a2a_in[b][hp][ds(130 * (2 * qs + 1) + 65 * t, 65), :],
                        on65[:, 256:512])
                fill(1)

            def do_a2a(b, h):
                nc.gpsimd.collective_compute(
                    "AllToAll", mybir.AluOpType.bypass,
                    replica_groups=[list(range(N_CORES))],
                    ins=[a2a_in[b][h].opt()], outs=[a2a_out[b][h].opt()])

            # ---- post-a2a: load attb, denominators, normalize ----
            rdbs = {}

            def oproj_load(b, h):
                av3 = a2a_out[b][h][:].rearrange("(j t e) n -> e j t n", j=8, t=2)
                ab3 = attb[b][:].rearrange("(t dh) (j hpj n) -> dh j hpj t n",
                                           t=2, dh=64, j=8, hpj=2)
                for t in range(2):
                    nc.sync.dma_start(ab3[:, :, h, t, :], av3[0:64, :, t, :])
                # denominators partition-major: [16(c,t), 256]; rows 64+65*i
                dsel = a2a_out[b][h][:].rearrange("(i s) n -> i s n", i=16)
                db = sp.tile([16, 256], bf, tag="db", bufs=1)
                nc.sync.dma_start(db[:], dsel[:, 64, :])
                dbf = sp.tile([16, 256], f32, tag="dbf", bufs=1)
                nc.vector.tensor_copy(dbf[:], db[:])
                rdf = sp.tile([16, 256], f32, tag="rdf", bufs=1)
                nc.vector.reciprocal_approx_fast(rdf[:], dbf[:])
                rdb = sp.tile([16, 256], bf, tag="rdb", name=f"rdb{b}{h}")
                nc.vector.tensor_copy(rdb[:], rdf[:])
                rdbs[(b, h)] = rdb

            def norm_attb(b, h):
                rdb = rdbs[(b, h)]
                ab = attb[b]
                for j in range(8):
                    k = 2 * j + h
                    bcd = smp.tile([128, 256], f32, tag="small", name=f"bcd{b}{h}{j}")
                    nc.tensor.matmul(bcd[:], selj[:, ts(j, 128)], rdb[:],
                                     start=True, stop=True)
                    nc.vector.tensor_mul(ab[:, ts(k, 256)], ab[:, ts(k, 256)], bcd[:])

            def oproj_mp(b, mp):  # full 16-k output projection for one m-pair
                psf = smp.tile([128, RC], f32, tag="small", name=f"psf{b}{mp}")
                for mi, m in enumerate((2 * mp, 2 * mp + 1)):
                    for k in range(16):
                        nc.tensor.matmul(psf[:, ds(mi * 256, 256)],
                                         wo[:, ds((m * 16 + k) * 128, 128)],
                                         attb[b][:, ts(k, 256)],
                                         start=(k == 0), stop=(k == 15))
                ofin = sp.tile([128, RC], f32, tag="ofin")
                nc.vector.tensor_copy(ofin[:], psf[:])
                for mi, m in enumerate((2 * mp, 2 * mp + 1)):
                    nc.sync.dma_start(out_ext[ts(m, 128), ds(b * 256, 256)],
                                      ofin[:, ds(mi * 256, 256)])

            def oproj_mp_even(mp):  # b1 even-k partial pass
                psf = smp.tile([128, RC], f32, tag="small", name=f"psfe{mp}")
                for mi, m in enumerate((2 * mp, 2 * mp + 1)):
                    for ki, k in enumerate(range(0, 16, 2)):
                        nc.tensor.matmul(psf[:, ds(mi * 256, 256)],
                                         wo[:, ds((m * 16 + k) * 128, 128)],
                                         attb[1][:, ts(k, 256)],
                                         start=(ki == 0), stop=(ki == 7))
                nc.vector.tensor_copy(OPART[:, ds(mp * 512, 512)], psf[:])

            def oproj_mp_odd(mp):  # b1 odd-k pass + combine + store
                psf = smp.tile([128, RC], f32, tag="small", name=f"psfo{mp}")
                for mi, m in enumerate((2 * mp, 2 * mp + 1)):
                    for ki, k in enumerate(range(1, 16, 2)):
                        nc.tensor.matmul(psf[:, ds(mi * 256, 256)],
                                         wo[:, ds((m * 16 + k) * 128, 128)],
                                         attb[1][:, ts(k, 256)],
                                         start=(ki == 0), stop=(ki == 7))
                ofin = sp.tile([128, RC], f32, tag="ofin")
                nc.vector.tensor_add(ofin[:], psf[:], OPART[:, ds(mp * 512, 512)])
                for mi, m in enumerate((2 * mp, 2 * mp + 1)):
                    nc.sync.dma_start(out_ext[ts(m, 128), ds(256, 256)],
                                      ofin[:, ds(mi * 256, 256)])

            # ================= emission schedule =================
            nc.sync.dma_start(wq[:, 0:1024], wq_sb[:, 0:1024])
            xt0 = xp.tile([128, NKT, RC], bf, tag="xt")
            nc.sync.dma_start(xt0[:, 0:4, :], x4[:, 0, 0:4, :])
            nc.sync.dma_start(wq[:, 1024:2048], wq_sb[:, 1024:2048])
            nc.sync.dma_start(xt0[:, 4:8, :], x4[:, 0, 4:8, :])
            nc.sync.dma_start(xt0[:, 8:12, :], x4[:, 0, 8:12, :])
            nc.sync.dma_start(xt0[:, 12:16, :], x4[:, 0, 12:16, :])
            cosc0 = sp.tile([128, RC], bf, tag="cos")
            nc.sync.dma_start(cosc0[:], cos_sb[:, 0:RC])
            sinc0 = sp.tile([128, RC], bf, tag="sin")
            nc.sync.dma_start(sinc0[:], sin_sb[:, 0:RC])
            st[0].update(xt=xt0, cos=cosc0, sin=sinc0)
            nc.sync.dma_start(wq[:, 2048:4096], wq_sb[:, 2048:4096])
            S0(1)
            nc.sync.dma_start(wkvt[:], wkv_sb[:])
            S1(0)
            for t_, s_ in ((ind, ind_sb), (indtq, indtq_sb), (indtk, indtk_sb),
                           (idn, idn_sb), (pswap, psw_sb), (tri, tri_sb),
                           (selj, selj_sb)):
                nc.sync.dma_start(t_[:], s_[:])
            S2(0); S3(0)
            nc.sync.dma_start(wo[:, ds(0, 8192)], wo_sb[:, ds(0, 8192)])
            nc.sync.dma_start(wo[:, ds(8192, 8192)], wo_sb[:, ds(8192, 8192)])
            S1(1); PS(0); S2(1); PB(0); S3(1); PW(0)
            nc.sync.dma_start(wo[:, ds(16384, 8192)], wo_sb[:, ds(16384, 8192)])
            S1(2); PS(1); S2(2); PB(1); S3(2); PW(1)
            nc.sync.dma_start(wo[:, ds(24576, 8192)], wo_sb[:, ds(24576, 8192)])
            S1(3); PS(2); S2(3); PB(2); S3(3); PW(2)

            # chunks 4-5 + chunk-3 norms interleave with attention (0,0,*)
            fill_q.extend([
                lambda: PS(3), lambda: S1(4), lambda: PB(3), lambda: S2(4),
                lambda: PW(3), lambda: S3(4), lambda: PS(4), lambda: S1(5),
                lambda: PB(4), lambda: S2(5), lambda: PW(4), lambda: S3(5),
                lambda: PS(5),
            ])
            for qs in range(4):
                attn_block(0, 0, qs)
            do_a2a(0, 0)

            fill_q.extend([
                lambda: PB(5), lambda: S1(6), lambda: PW(5), lambda: S2(6),
                lambda: S3(6), lambda: PS(6), lambda: PB(6), lambda: PW(6),
            ])
            for qs in range(4):
                attn_block(0, 1, qs)
            do_a2a(0, 1)

            fill_q.extend([
                lambda: S1(7), lambda: oproj_load(0, 0), lambda: S2(7),
                lambda: S3(7), lambda: PS(7), lambda: PB(7), lambda: PW(7),
                lambda: norm_attb(0, 0),
                lambda: oproj_load(0, 1), lambda: norm_attb(0, 1),
                lambda: oproj_mp(0, 0), lambda: oproj_mp(0, 1),
                lambda: oproj_mp(0, 2), lambda: oproj_mp(0, 3),
            ])
            for qs in range(4):
                attn_block(1, 0, qs)
            do_a2a(1, 0)

            fill_q.extend([
                lambda: oproj_load(1, 0), lambda: norm_attb(1, 0),
                lambda: oproj_mp_even(0), lambda: oproj_mp_even(1),
                lambda: oproj_mp_even(2),
            ])
            for qs in range(4):
                attn_block(1, 1, qs)
            do_a2a(1, 1)
            fill(len(fill_q))  # drain leftovers
            oproj_load(1, 1)   # DMAs + recip chain run during a2a flight
            for mp_ in (3, 4, 5, 6, 7):
                oproj_mp_even(mp_)
            oproj_mp(0, 4)
            oproj_mp(0, 5)
            oproj_mp(0, 6)
            oproj_mp(0, 7)
            if dbg2_ext is not None:
                nc.sync.dma_start(dbg2_ext[:, ds(512, 256)], a2a_out[0][0][0:128, :])
                nc.sync.dma_start(dbg2_ext[:, ds(768, 256)], a2a_out[0][0][130:258, :])
            if dbg_ext is not None:
                o = 0
                nc.sync.dma_start(dbg_ext[:, ds(o, 16 * 65)], Vb1[0][:]); o += 16 * 65
                nc.sync.dma_start(dbg_ext[0:16, ds(o, 256)], rdbs[(0, 0)][:]); o += 256
                nc.sync.dma_start(dbg_ext[0:16, ds(o, 256)], rdbs[(0, 1)][:]); o += 256
                nc.sync.dma_start(dbg_ext[0:16, ds(o, 256)], rdbs[(1, 0)][:]); o += 2 * 256
                nc.sync.dma_start(dbg_ext[:, ds(o, 16 * 256)], attb[0][:]); o += 16 * 256
                nc.sync.dma_start(dbg_ext[:, ds(o, ROWS)], QTn[0][:])
            norm_attb(1, 1)
            for mp in range(8):
                oproj_mp_odd(mp)

    nc.compile()
    return nc


def _host_prep(x, freqs_cos, freqs_sin, wq, wk, wv, wo, q_scale, k_scale):
    bfd = ml_dtypes.bfloat16
    perm = np.concatenate([np.arange(0, 64, 2), np.arange(1, 64, 2)])

    xT = np.ascontiguousarray(x.reshape(ROWS, D).T)  # [D, ROWS]
    # chunk-contiguous: [128, chunk, k, RC]
    x_sb = np.ascontiguousarray(
        xT.reshape(NKT, 128, NCHUNK, RC).transpose(1, 2, 0, 3)
        .reshape(128, NCHUNK * NKT * RC)
    ).astype(bfd)

    ct = np.concatenate([freqs_cos.T, freqs_cos.T], axis=1)   # [32, 4096]
    st = np.concatenate([freqs_sin.T, freqs_sin.T], axis=1)
    cos_sb = np.ascontiguousarray(np.tile(ct, (4, 1))).astype(bfd)
    sin_sb = np.ascontiguousarray(np.concatenate([-st, st, -st, st], 0)).astype(bfd)

    r = np.arange(128)[:, None]
    c = np.arange(128)[None, :]
    tri_sb = (c >= r).astype(bfd)
    ind_sb = np.zeros((128, 2), bfd)
    ind_sb[0:64, 0] = 1
    ind_sb[64:128, 1] = 1
    qs_p = q_scale[perm].astype(np.float32)
    ks_p = k_scale[perm].astype(np.float32)
    indtq_sb = np.zeros((2, 128), np.float32)
    indtq_sb[0, 0:64] = qs_p
    indtq_sb[1, 64:128] = qs_p
    indtq_sb = indtq_sb.astype(bfd)
    indtk_sb = ks_p.reshape(1, 64).astype(bfd)
    selj_sb = np.zeros((16, 8 * 128), np.float32)
    for j in range(8):
        selj_sb[2 * j, j * 128:j * 128 + 64] = 1
        selj_sb[2 * j + 1, j * 128 + 64:j * 128 + 128] = 1
    selj_sb = selj_sb.astype(bfd)
    idn_sb = np.eye(64, dtype=bfd)
    psw_np = np.zeros((128, 128), np.float32)
    g = np.arange(128)
    psw_np[g, (g // 32 ^ 1) * 32 + g % 32] = 1.0
    psw_sb = psw_np.astype(bfd)

    woT = wo.T.astype(np.float32)  # [hdim, dout]
    wo_sb = np.ascontiguousarray(
        woT.reshape(16, 128, 16, 128).transpose(1, 2, 0, 3).reshape(128, 16 * 16 * 128)
    ).astype(bfd)

    shared = dict(x_sb=x_sb, cos_sb=cos_sb, sin_sb=sin_sb, tri_sb=tri_sb,
                  ind_sb=ind_sb, indtq_sb=indtq_sb, indtk_sb=indtk_sb,
                  selj_sb=selj_sb, idn_sb=idn_sb, psw_sb=psw_sb, wo_sb=wo_sb)

    in_maps = []
    for cc in range(N_CORES):
        wq_c = wq[cc * 256:(cc + 1) * 256].reshape(4, 64, D)[:, perm].reshape(256, D)
        wqT = wq_c.T  # [D, 256]
        wq_core = np.ascontiguousarray(
            wqT.reshape(NKT, 128, 2, 128).transpose(1, 2, 0, 3).reshape(128, NKT * 256)
        ).astype(bfd)
        wk_c = wk[cc * 64:(cc + 1) * 64][perm]
        wv_c = wv[cc * 64:(cc + 1) * 64]
        wkvT = np.concatenate([wk_c, wv_c], 0).T  # [D, 128]: cols 0:64=K(perm), 64:128=V
        wkv_core = np.ascontiguousarray(
            wkvT.reshape(NKT, 128, 128).transpose(1, 0, 2).reshape(128, NKT * 128)
        ).astype(bfd)
        in_maps.append(dict(shared, wq_sb=wq_core, wkv_sb=wkv_core))
    return in_maps


def kernel(x, freqs_cos, freqs_sin, wq, wk, wv, wo, q_scale, k_scale, _trace=False,
           _debug=False):
    from concourse.bass_utils import run_bass_kernel_spmd

    key = "ncdbg" if _debug else "nc"
    if key not in _cache:
        _cache[key] = _build(_DEBUG=_debug)
    nc = _cache[key]

    args = [np.asarray(a, dtype=np.float32) for a in
            (x, freqs_cos, freqs_sin, wq, wk, wv, wo, q_scale, k_scale)]
    in_maps = _host_prep(*args)
    res = run_bass_kernel_spmd(nc, in_maps, list(range(N_CORES)), trace=_trace)
    out = np.zeros((B, S, D), np.float32)
    for cc in range(N_CORES):
        oc = res.results[cc]["out"]  # [2048, 512]
        for b in range(B):
            out[b, 256 * cc:256 * (cc + 1), :] = oc[:, 256 * b:256 * (b + 1)].T
    if _debug:
        return out, res
    if _trace:
        return out, res
    return out
